# revision 18
# baseline (speedup 1.0000x reference)
"""Trainium2 Bass kernel for nn_PartialConvLayer (partial conv 3x3 + mask
update + BatchNorm(batch stats) + ReLU), data-parallel over batch on 8 cores.

Math (per image):
  update = conv(mask, ones(Cin,3,3)), pad 1          # integer in {0..576}
  u      = clip(update, 0, 1)                        # exactly binary
  mr     = 576 / (update + 1e-6) * u
  conv   = conv(x*mask, W), pad 1                    # no bias
  out    = conv * mr + b * u
         = (conv + (b/576) (x) v) * mr,  v = u*(update+1e-6)   [u^2 == u]
  BN over (N,H,W) batch stats (all-reduced across cores), then ReLU.
Returns (out, broadcast(update_clipped)).

Perf design:
  - bf16 activations/weights/outputs (tolerance 2e-2; bf16 error ~0.5%).
  - One DMA instruction per tensor per block for input loads, with the
    64-count channel dim outermost so descriptors spray across all DMA
    rings (the DGE sprays on the outermost AP dim). DMA instruction issue
    costs ~600ns of sequencer time, so instruction count is minimized
    everywhere: strips/reshapes are merged via an interleaved row order
    (partition 4*(j//2)+2*band+(j%2)) so each is a single DMA.
  - Pre-BN activations stay resident in SBUF as bf16 (128 KB/partition);
    pass 2 reads SBUF and writes bf16 DRAM.
  - band0 (rows 0..63) / band1 (rows 64..127) conv matmuls issued
    back-to-back so they run concurrently on different PE row groups.
  - Block k+1's loads are issued before block k's compute (prefetch) to
    keep the PE dense and clock-warm.
"""
import os
import numpy as np
from contextlib import ExitStack

import ml_dtypes

import concourse.bass as bass
import concourse.tile as tile
from concourse import mybir, bacc
from concourse import library_config
from concourse.bass_utils import run_bass_kernel_spmd

F32 = mybir.dt.float32
BF16 = mybir.dt.bfloat16
ALU = mybir.AluOpType
ACTF = mybir.ActivationFunctionType

CIN = 64
COUT = 128
W_ = 256          # image width
KS = 3
EPS_MASK = 1e-6
EPS_BN = 1e-5
SLIDE = float(CIN * KS * KS)   # 576
NPBF = ml_dtypes.bfloat16


def build_nc(n_cores=8, H=256, B=8):
    """SPMD program for one core holding one [CIN, H, W_] image."""
    HB = H // 2                      # rows per band
    nblk = HB // B                   # blocks
    nrows = B + 2                    # rows per band tile (with halo)
    npair = nrows // 2               # row-pairs for the s matmuls
    nchunk = (H * W_) // 512         # 512-px chunks per core
    TOT = float(n_cores * H * W_)    # BN count
    HW = H * W_
    NJ = B // 2                      # j-pairs (chunk pairs) per block

    nc = bacc.Bacc(None, num_devices=n_cores)

    # x/mask pre-split into bands on host: partition-row b*64+c holds rows
    # (b*HB-1 .. b*HB+HB) of channel c, zero-padded outside the image.
    X = nc.dram_tensor("x", [128, (HB + 2) * W_], BF16, kind="ExternalInput")
    M = nc.dram_tensor("mask", [128, (HB + 2) * W_], BF16, kind="ExternalInput")
    WT = nc.dram_tensor("wt", [128, KS * KS * COUT], BF16, kind="ExternalInput")
    BP2 = nc.dram_tensor("bp2", [128, COUT], BF16, kind="ExternalInput")
    ONES2 = nc.dram_tensor("ones2", [128, 2], BF16, kind="ExternalInput")
    T3 = nc.dram_tensor("t3", [2 * nrows, 2 * B], BF16, kind="ExternalInput")
    GAM = nc.dram_tensor("gam", [COUT, 1], F32, kind="ExternalInput")
    BET = nc.dram_tensor("bet", [COUT, 1], F32, kind="ExternalInput")

    OUT = nc.dram_tensor("out", [COUT, HW], BF16, kind="ExternalOutput")
    UPD = nc.dram_tensor("upd", [H, W_], F32, kind="ExternalOutput")

    cc_in = nc.dram_tensor("ccin", [COUT, 2], F32)
    cc_out = nc.dram_tensor("ccout", [COUT, 2], F32,
                            addr_space="Shared" if n_cores > 4 else "Local")

    with tile.TileContext(nc) as tc, ExitStack() as ctx:
        nc.gpsimd.load_library(library_config.mlp)

        const = ctx.enter_context(tc.tile_pool(name="const", bufs=1))
        io = ctx.enter_context(tc.tile_pool(name="io", bufs=2))
        sblk = ctx.enter_context(tc.tile_pool(name="sblk", bufs=1))
        updp = ctx.enter_context(tc.tile_pool(name="updp", bufs=2))
        strp = ctx.enter_context(tc.tile_pool(name="strp", bufs=2))
        sqp = ctx.enter_context(tc.tile_pool(name="sqp", bufs=1))
        stp1 = ctx.enter_context(tc.tile_pool(name="stp1", bufs=1))
        p2p = ctx.enter_context(tc.tile_pool(name="p2p", bufs=3))
        psc = ctx.enter_context(tc.tile_pool(name="psc", bufs=2, space="PSUM"))
        pss = ctx.enter_context(tc.tile_pool(name="pss", bufs=1, space="PSUM"))
        psu = ctx.enter_context(tc.tile_pool(name="psu", bufs=2, space="PSUM"))

        # ---- constants ----
        wt_t = const.tile([128, KS * KS * COUT], BF16)
        nc.sync.dma_start(wt_t[:], WT[:])
        bp_t = const.tile([128, COUT], BF16)
        nc.sync.dma_start(bp_t[:], BP2[:])
        ones2_t = const.tile([128, 2], BF16)
        nc.sync.dma_start(ones2_t[:], ONES2[:])
        t3_t = const.tile([2 * nrows, 2 * B], BF16)
        nc.sync.dma_start(t3_t[:], T3[:])
        gam_t = const.tile([COUT, 1], F32)
        nc.sync.dma_start(gam_t[:], GAM[:])
        bet_t = const.tile([COUT, 1], F32)
        nc.sync.dma_start(bet_t[:], BET[:])
        eps_t = const.tile([COUT, 1], F32)
        nc.vector.memset(eps_t[:], EPS_BN)
        sum_slots = const.tile([COUT, nchunk], F32)
        sq_slots = const.tile([COUT, nchunk], F32)
        # pre-BN activations, SBUF-resident for the whole kernel (bf16)
        prebn = const.tile([COUT, HW], BF16)
        # persistent padded xm buffers; guard cols 0/257 zeroed once
        xm_tiles = []
        for i in range(2):
            t = const.tile([128, nrows * 258], BF16, tag=f"xm{i}")
            nc.vector.memset(t[:], 0.0)
            xm_tiles.append(t)
        # persistent s_rows buffers; guard cols 0/257 zeroed once
        sr_tiles = []
        for i in range(2):
            t = const.tile([2 * nrows, 258], BF16, tag=f"sr{i}")
            nc.vector.memset(t[:], 0.0)
            sr_tiles.append(t)

        def issue_loads(k):
            """Prefetch block k's x/mask band tiles (halo rows included)."""
            r0 = k * B
            x_t = io.tile([128, nrows * W_], BF16, tag="x_t")
            m_t = io.tile([128, nrows * W_], BF16, tag="m_t")
            for tens, tl, eng in ((X, x_t, nc.sync), (M, m_t, nc.scalar)):
                eng.dma_start(
                    tl[:, :],
                    bass.AP(tensor=tens, offset=r0 * W_,
                            ap=[[(HB + 2) * W_, 128], [1, nrows * W_]]))
            return x_t, m_t

        ci_global = 0

        def make_state(k):
            x_t, m_t = issue_loads(k)
            st = {"k": k, "x_t": x_t, "m_t": m_t}
            st["x3"] = x_t[:, :].rearrange("p (r c) -> p r c", c=W_)
            st["m3"] = m_t[:, :].rearrange("p (r c) -> p r c", c=W_)
            xm = xm_tiles[k % 2]
            st["xm3"] = xm[:, :].rearrange("p (r c) -> p r c", c=258)
            return st

        # s-matmul row groups: pairs of 2 rows (fp32 PSUM limit = 512)
        sgroups = [(2 * p, 2) for p in range(npair)]

        def prep_steps(st):
            """Mask-prep for block st[k], yielded in 4 chunks to interleave
            into the previous block's conv j-loop."""
            k = st["k"]
            # -- step 0: xm product + first s matmul --
            nc.vector.tensor_tensor(st["xm3"][:, :, 1:257], st["x3"],
                                    st["m3"], op=ALU.mult)
            s_all = sblk.tile([2, 2 * npair * 256], BF16, tag="s_all")
            st["s_all"] = s_all

            def smm(gi):
                r, g = sgroups[gi]
                ps_s = pss.tile([2, g * 256], F32, tag="ps_s")
                nc.tensor.matmul(ps_s[:], ones2_t[:],
                                 st["m3"][:, r:r + g, :], start=True,
                                 stop=True)
                nc.scalar.copy(s_all[:, r * 256:(r + g) * 256], ps_s[:])

            smm(0)
            smm(1)
            yield
            # -- step 1: remaining s matmuls + reshape --
            for gi in range(1, len(sgroups)):
                smm(gi)
            s_rows = sr_tiles[k % 2]
            st["s_rows"] = s_rows
            nc.scalar.dma_start(
                s_rows[:, 1:257],
                s_all[:, :].rearrange("b (r f) -> b r f", f=256))
            yield
            # -- step 2: T3 + horizontal sums + reciprocal --
            ps_u = psu.tile([2 * B, 258], F32, tag="ps_u")
            nc.tensor.matmul(ps_u[:], t3_t[:], s_rows[:, :], start=True,
                             stop=True)
            u_sb = updp.tile([2 * B, 258], F32, tag="u_sb")
            nc.scalar.copy(u_sb[:], ps_u[:])
            vh = updp.tile([2 * B, W_], F32, tag="vh")
            nc.vector.tensor_add(vh[:], u_sb[:, 0:256], u_sb[:, 1:257])
            nc.vector.tensor_add(vh[:], vh[:], u_sb[:, 2:258])
            u_clip = updp.tile([2 * B, W_], F32, tag="u_clip")
            nc.vector.tensor_scalar_min(u_clip[:], vh[:], 1.0)
            nc.vector.tensor_scalar_add(vh[:], vh[:], EPS_MASK)  # vh -> upde
            rec = updp.tile([2 * B, W_], F32, tag="rec")
            nc.vector.reciprocal(rec[:], vh[:])
            st["u_clip"] = u_clip
            st["vh"] = vh
            st["rec"] = rec
            yield
            # -- step 3: mru/v rows, UPD out, mru strip --
            mru_rows = updp.tile([2 * B, W_], BF16, tag="mru_rows")
            nc.vector.scalar_tensor_tensor(
                out=mru_rows[:], in0=st["rec"][:], scalar=SLIDE,
                in1=st["u_clip"][:], op0=ALU.mult, op1=ALU.mult)
            v_rows = updp.tile([2 * B, W_], BF16, tag="v_rows")
            nc.vector.scalar_tensor_tensor(
                out=v_rows[:], in0=st["vh"][:], scalar=1.0,
                in1=st["u_clip"][:], op0=ALU.mult, op1=ALU.mult)
            nc.scalar.dma_start(
                bass.AP(tensor=UPD, offset=st["k"] * B * W_,
                        ap=[[HB * W_, 2], [1, B * W_]]),
                st["u_clip"][:, :])
            mst = stp1.tile([1, 2 * B * W_], BF16, tag="mst")
            nc.sync.dma_start(mst[:, :], mru_rows[:, :])
            st["mru_rows"] = mru_rows
            st["v_rows"] = v_rows
            st["mst"] = mst
            yield

        def conv_block(st, next_steps):
            nonlocal ci_global
            k = st["k"]
            r0 = k * B
            xm3 = st["xm3"]
            v_rows = st["v_rows"]
            mst = st["mst"]
            for q in range(NJ):
                j = 2 * q
                vst = strp.tile([128, 512], BF16, tag="vst")
                nc.sync.dma_start(vst[0:1, :], v_rows[j:j + 2, :])
                nc.scalar.dma_start(vst[64:65, :], v_rows[B + j:B + j + 2, :])
                mru_bc = strp.tile([128, 1024], BF16, tag="mru_bc")
                nc.gpsimd.partition_broadcast(
                    mru_bc[:, 0:512], mst[0:1, 256 * j:256 * j + 512])
                nc.gpsimd.partition_broadcast(
                    mru_bc[:, 512:1024],
                    mst[0:1, 256 * (B + j):256 * (B + j) + 512])

                ps_c0 = psc.tile([COUT, 512], F32, tag="ps_c0")
                ps_c1 = psc.tile([COUT, 512], F32, tag="ps_c1")
                for t in range(KS * KS):
                    ky, kx = divmod(t, KS)
                    nc.tensor.matmul(
                        ps_c0[:], wt_t[0:64, t * COUT:(t + 1) * COUT],
                        xm3[0:64, j + ky:j + ky + 2, kx:kx + 256],
                        start=(t == 0), stop=False)
                    nc.tensor.matmul(
                        ps_c1[:], wt_t[64:128, t * COUT:(t + 1) * COUT],
                        xm3[64:128, j + ky:j + ky + 2, kx:kx + 256],
                        start=(t == 0), stop=False)
                nc.tensor.matmul(ps_c0[:], bp_t[0:1, :], vst[0:1, :],
                                 start=False, stop=True)
                nc.tensor.matmul(ps_c1[:], bp_t[64:65, :], vst[64:65, :],
                                 start=False, stop=True)

                for b, ps_c in ((0, ps_c0), (1, ps_c1)):
                    off = (b * HB + r0 + j) * W_
                    ci = ci_global + b
                    pslice = prebn[:, off:off + 512]
                    nc.vector.scalar_tensor_tensor(
                        out=pslice, in0=ps_c[:], scalar=0.0,
                        in1=mru_bc[:, 512 * b:512 * b + 512],
                        op0=ALU.add, op1=ALU.mult,
                        accum_out=sum_slots[:, ci:ci + 1])
                    sq_scr = sqp.tile([COUT, 512], BF16, tag="sq_scr")
                    nc.scalar.activation(
                        sq_scr[:], pslice, ACTF.Square,
                        accum_out=sq_slots[:, ci:ci + 1])
                ci_global += 2
                # drive the next block's mask-prep between j-groups
                if next_steps is not None:
                    next(next_steps, None)

        st_cur = make_state(0)
        steps0 = prep_steps(st_cur)
        for _ in steps0:
            pass
        for k in range(nblk):
            if k + 1 < nblk:
                st_next = make_state(k + 1)
                nsteps = prep_steps(st_next)
            else:
                st_next, nsteps = None, None
            conv_block(st_cur, nsteps)
            if nsteps is not None:
                for _ in nsteps:   # drain any remaining steps
                    pass
            st_cur = st_next

        assert ci_global == nchunk

        # ---- BN stats: reduce, all-reduce, affine coeffs ----
        cc_sb = const.tile([COUT, 2], F32)
        nc.vector.tensor_reduce(cc_sb[:, 0:1], sum_slots[:],
                                axis=mybir.AxisListType.X, op=ALU.add)
        nc.vector.tensor_reduce(cc_sb[:, 1:2], sq_slots[:],
                                axis=mybir.AxisListType.X, op=ALU.add)
        nc.sync.dma_start(cc_in[:], cc_sb[:])
        nc.gpsimd.collective_compute(
            "AllReduce", ALU.add,
            replica_groups=[list(range(n_cores))],
            ins=[cc_in.ap().opt()], outs=[cc_out.ap().opt()])
        st_sb = const.tile([COUT, 2], F32)
        nc.sync.dma_start(st_sb[:], cc_out[:])
        mean_t = const.tile([COUT, 1], F32)
        nc.vector.tensor_scalar_mul(mean_t[:], st_sb[:, 0:1], 1.0 / TOT)
        e2_t = const.tile([COUT, 1], F32)
        nc.vector.tensor_scalar_mul(e2_t[:], st_sb[:, 1:2], 1.0 / TOT)
        msq_t = const.tile([COUT, 1], F32)
        nc.vector.tensor_mul(msq_t[:], mean_t[:], mean_t[:])
        var_t = const.tile([COUT, 1], F32)
        nc.vector.tensor_sub(var_t[:], e2_t[:], msq_t[:])
        std_t = const.tile([COUT, 1], F32)
        nc.scalar.activation(std_t[:], var_t[:], ACTF.Sqrt, bias=eps_t[:])
        rstd_t = const.tile([COUT, 1], F32)
        nc.vector.reciprocal(rstd_t[:], std_t[:])
        scale_t = const.tile([COUT, 1], F32)
        nc.vector.tensor_mul(scale_t[:], gam_t[:], rstd_t[:])
        tmp_t = const.tile([COUT, 1], F32)
        nc.vector.tensor_mul(tmp_t[:], mean_t[:], scale_t[:])
        bias_t = const.tile([COUT, 1], F32)
        nc.vector.tensor_sub(bias_t[:], bet_t[:], tmp_t[:])

        # ---- pass 2: out = relu(scale*prebn + bias), split ACT/DVE ----
        P2 = 2048
        n2 = HW // P2
        for i2 in range(n2):
            i = i2 * P2
            o_t = p2p.tile([COUT, P2], BF16, tag="o_t")
            if (i2 % 12) in (0, 2, 5, 7, 10):
                nc.scalar.activation(o_t[:], prebn[:, i:i + P2], ACTF.Relu,
                                     bias=bias_t[:], scale=scale_t[:])
            else:
                nc.vector.tensor_scalar(o_t[:], prebn[:, i:i + P2],
                                        scale_t[:], bias_t[:],
                                        op0=ALU.mult, op1=ALU.add)
                nc.vector.tensor_scalar_max(o_t[:], o_t[:], 0.0)
            eng = nc.sync if i2 % 2 == 0 else nc.scalar
            eng.dma_start(OUT[:, i:i + P2], o_t[:])

    return nc


def make_host_inputs(x_i, mask_i, W, b, gamma, beta, B=8):
    """Per-core in_map for one image shard (host-side constant prep)."""
    nrows = B + 2
    WT1 = np.ascontiguousarray(
        W.transpose(1, 2, 3, 0).reshape(CIN, KS * KS * COUT))
    WT = np.concatenate([WT1, WT1], axis=0).astype(NPBF)
    BP2 = np.zeros((128, COUT), NPBF)
    BP2[0, :] = (b / SLIDE).astype(NPBF)
    BP2[64, :] = (b / SLIDE).astype(NPBF)
    ones2 = np.zeros((128, 2), NPBF)
    ones2[0:64, 0] = 1.0
    ones2[64:128, 1] = 1.0
    T3 = np.zeros((2 * nrows, 2 * B), NPBF)
    for band in range(2):
        for jj in range(B):
            for d in range(3):
                T3[band * nrows + jj + d, band * B + jj] = 1.0
    def band_split(a):
        """[CIN, H, W] -> [128, (HB+2)*W]: rows b*HB-1..b*HB+HB, zero-padded."""
        CINL, H, W = a.shape
        HB = H // 2
        ap = np.zeros((CINL, H + 2, W), a.dtype)
        ap[:, 1:H + 1] = a
        out = np.empty((2, CINL, HB + 2, W), a.dtype)
        for b in range(2):
            out[b] = ap[:, b * HB:b * HB + HB + 2]
        return np.ascontiguousarray(
            out.transpose(0, 1, 2, 3).reshape(2 * CINL, (HB + 2) * W))

    return {
        "x": band_split(np.ascontiguousarray(x_i).astype(NPBF)),
        "mask": band_split(np.ascontiguousarray(mask_i).astype(NPBF)),
        "wt": WT,
        "bp2": BP2,
        "ones2": ones2,
        "t3": T3,
        "gam": gamma.reshape(COUT, 1).astype(np.float32),
        "bet": beta.reshape(COUT, 1).astype(np.float32),
    }


_NC_CACHE = {}


def kernel(x, mask, W, b, gamma, beta):
    x = np.asarray(x)
    mask = np.asarray(mask)
    W = np.asarray(W)
    b = np.asarray(b)
    gamma = np.asarray(gamma)
    beta = np.asarray(beta)
    N, _, H, _ = x.shape
    n_cores = N
    key = (n_cores, H)
    if key not in _NC_CACHE:
        nc = build_nc(n_cores=n_cores, H=H)
        nc.finalize()
        _NC_CACHE[key] = nc
    nc = _NC_CACHE[key]

    in_maps = [make_host_inputs(x[i], mask[i], W, b, gamma, beta)
               for i in range(n_cores)]
    res = run_bass_kernel_spmd(nc, in_maps, core_ids=list(range(n_cores)),
                               trace=bool(os.environ.get("KERNEL_TRACE")))
    out = np.stack([res.results[i]["out"].astype(np.float32)
                    .reshape(COUT, H, W_) for i in range(n_cores)])
    upd = np.stack([res.results[i]["upd"] for i in range(n_cores)])
    update_full = np.broadcast_to(upd[:, None, :, :], (N, COUT, H, W_))
    kernel.last_result = res
    return out, update_full
